# revision 1
# baseline (speedup 1.0000x reference)
"""Trainium2 Bass kernel for ContextAwareAttention (fp8 DoubleRow version).

Math (per batch row b):
    pi  = x[b] @ W_in.T + b_in                  # [S, D]
    pc  = context[b] @ W_ctx.T + b_ctx          # [D]   (host-precomputed)
    h   = tanh(pi + pc)                         # [S, D]
    sc  = h @ w_att (+ b_att, dropped: softmax shift-invariant)   # [S]
    w   = softmax(sc)                           # [S]
    out = w @ x[b]                              # [D]

Sharding: data-parallel over batch, 2 batch rows per NeuronCore x 8 cores.

Device layout (per core), e-on-partitions for pass 1:
  - Pass-1 matmuls in fp8e4 with perf_mode=DoubleRow: psum[128e, 512s] +=
    sum_cc W8[h,cc].T(x2-packed) @ x8[cc](x2-packed), K=256 per instruction.
    W is pre-scaled by 16 host-side so its fp8 mantissa is fully used; the
    tanh activation applies scale=1/16 and the per-partition bias pc[b,e]
    in the same ScalarE op (no separate DVE bias add).
  - Scores: sc[1, 512s] += w_att_chunk[128,1].T @ tanh_tile[128,512] on PE
    (8 accumulating matmuls per s-block, pipelined one e-chunk behind tanh).
  - Softmax WITHOUT max subtraction (scores are bounded, |sc| <~ 2, so
    exp stays deep inside f32/bf16 range): per-block fused exp+sum via
    accum_out as soon as a block's scores finish. The 1/Z normalization
    happens on HOST (exact); device ships unnormalized out_un and Z.
    This removes the all-blocks -> max -> exp serialization, so exp /
    broadcast / pass-2 pipeline inside pass 1 per 512-column block.
  - p broadcast to 128 partitions via K=1 matmul with a ones lhsT.
  - Pass 2 on DVE (mul+reduce) / GpSimd (mul) / ScalarE (tail reduces)
    against a resident bf16 x^T copy, chunk-scheduled behind the pbb
    broadcasts; only the last 512-column slice is tail-exposed.
  - PE warm-up matmuls during the initial DMA window (HAM clock-gate).
  - NOTE: custom DVE ISA ops (tensor_tensor_reduce / scalar_tensor_tensor)
    compile but fault at runtime in this environment -- native ops only.
"""

import os
import numpy as np
import ml_dtypes

BF16 = ml_dtypes.bfloat16
FP8 = ml_dtypes.float8_e4m3      # TRN FP8_EXP4: bias 7, max +-240

P = 128          # partitions
D = 1024         # hidden dim
S = 4096         # sequence length per batch row
B_FULL = 16      # full batch
N_CORES = 8
B_LOC = B_FULL // N_CORES        # batch rows per core (2)
M = B_LOC * S                    # s per core (8192)
NB = 512                         # s-columns per block (psum free dim)
NH = D // P                      # e-chunks (8)
NCC = D // (2 * P)               # d chunk-pairs for DoubleRow (4)
QW = 2048                        # s-columns per resident x8 tile
WSCALE = 16.0                    # host premultiplier on W_in before fp8

_BUILT = {}


def _build(m_loc=M, b_loc=B_LOC, reps=1):
    import concourse.bass as bass  # noqa: F401
    import concourse.tile as tile
    from concourse import mybir, bacc
    from contextlib import ExitStack

    dt = mybir.dt
    DR = mybir.MatmulPerfMode.DoubleRow
    s_loc = m_loc // b_loc               # sequence per batch row
    nq = m_loc // QW                     # x8 tiles per cc
    nblk = s_loc // NB                   # s-blocks per batch row
    QP = 2 * NB                          # pass-2 chunk width (1024)
    nj = s_loc // QP                     # pass-2 chunks per d-chunk per row

    nc = bacc.Bacc("TRN2", target_bir_lowering=False, debug=False)

    x8_t = nc.dram_tensor("x8", [NCC, P, 2, m_loc], dt.float8e4, kind="ExternalInput")
    xbf_t = nc.dram_tensor("xbf", [D, m_loc], dt.bfloat16, kind="ExternalInput")
    w8_t = nc.dram_tensor("w8", [P, NH * NCC, 2, P], dt.float8e4, kind="ExternalInput")
    pcb_t = nc.dram_tensor("pcb", [P, NH, b_loc], dt.float32, kind="ExternalInput")
    wat_t = nc.dram_tensor("wat", [P, NH], dt.bfloat16, kind="ExternalInput")
    out_t = nc.dram_tensor("out", [b_loc, D], dt.float32, kind="ExternalOutput")
    z_t = nc.dram_tensor("z", [1, b_loc], dt.float32, kind="ExternalOutput")

    xbf_r = xbf_t[:].rearrange("(c p) s -> c p s", p=P)    # [8, 128, m_loc]
    out_r = out_t[:].rearrange("b (c q) -> b q c", q=P)    # [b_loc, 128, 8]

    with tile.TileContext(nc) as tc, ExitStack() as ctx:
        const = ctx.enter_context(tc.tile_pool(name="const", bufs=1))
        xres = ctx.enter_context(tc.tile_pool(name="xres", bufs=1))
        hpool = ctx.enter_context(tc.tile_pool(name="hs", bufs=3))
        spool = ctx.enter_context(tc.tile_pool(name="small", bufs=2))
        jpool = ctx.enter_context(tc.tile_pool(name="junk", bufs=3))
        drp = ctx.enter_context(tc.tile_pool(name="drp", bufs=3, space="PSUM"))
        scp = ctx.enter_context(tc.tile_pool(name="scp", bufs=3, space="PSUM"))
        bcp = ctx.enter_context(tc.tile_pool(name="bcp", bufs=2, space="PSUM"))

        # ---- constants / weights ----
        w8sb = const.tile([P, NH * NCC, 2, P], dt.float8e4)
        for i in range(4):
            nc.scalar.dma_start(out=w8sb[:, i * NCC * 2:(i + 1) * NCC * 2],
                                in_=w8_t[:][:, i * NCC * 2:(i + 1) * NCC * 2])
        pcb = const.tile([P, NH, b_loc], dt.float32)
        nc.gpsimd.dma_start(out=pcb, in_=pcb_t[:])
        wat = const.tile([P, NH], dt.bfloat16)
        nc.gpsimd.dma_start(out=wat, in_=wat_t[:])

        # ---- persistent row-shared buffers ----
        pbb = const.tile([P, s_loc], dt.bfloat16)
        zout = const.tile([1, b_loc], dt.float32)
        ones1 = const.tile([1, P], dt.bfloat16)
        nc.vector.memset(ones1, 1.0)

        # ---- PE warm-up: ~3.5us of dummy matmuls during the initial DMA
        # window so the HAM clock-gate reaches 8/8 before real work.
        # memset source: no dependency on any inbound DMA ----
        wu_src = const.tile([P, 2 * P], dt.bfloat16)
        nc.vector.memset(wu_src, 0.125)
        wu_ps = bcp.tile([P, 2 * P], dt.float32, tag="bc", name="bc")
        for i in range(14):
            nc.tensor.matmul(wu_ps, lhsT=wu_src[:, 0:P], rhs=wu_src,
                             start=(i == 0), stop=(i == 13))

        for rep in range(reps):
            # ---- resident fp8 x tiles: [cc][q] of [128, 2, QW] ----
            x8sb = [[None] * nq for _ in range(NCC)]
            for cc in range(NCC):
                for q in range(nq):
                    x8sb[cc][q] = xres.tile([P, 2, QW], dt.float8e4,
                                            tag=f"x8_{cc}_{q}",
                                            name=f"x8_{cc}_{q}")
            # block 0's operands first; remaining x8 interleaved with the xbf
            # streams below (queue order == emission order == priority)
            for cc in range(NCC):
                nc.sync.dma_start(out=x8sb[cc][0][:, :, 0:NB],
                                  in_=x8_t[cc][:, :, 0:NB])
            for cc in range(NCC):
                nc.sync.dma_start(out=x8sb[cc][0][:, :, NB:QW],
                                  in_=x8_t[cc][:, :, NB:QW])
            for cc in range(NCC):
                nc.sync.dma_start(out=x8sb[cc][1],
                                  in_=x8_t[cc][:, :, QW:2 * QW])

            for b in range(b_loc):
                # resident bf16 x for pass 2 of this row (tag-reuse across rows)
                xbf = []
                for c in range(NH):
                    t = xres.tile([P, s_loc], dt.bfloat16, tag=f"xbf{c}",
                                  name=f"xbf{c}")
                    xbf.append(t)
                hw_ = s_loc // 2
                # halves separately: subtile WAR lets the next row's first
                # half load as soon as this row's early chunks have read it
                for c in range(NH):
                    nc.sync.dma_start(out=xbf[c][:, 0:hw_],
                                      in_=xbf_r[c][:, b * s_loc:
                                                   b * s_loc + hw_])
                for c in range(NH):
                    nc.sync.dma_start(out=xbf[c][:, hw_:s_loc],
                                      in_=xbf_r[c][:, b * s_loc + hw_:
                                                   (b + 1) * s_loc])
                if b == 0 and nq > 2:
                    # second batch row's fp8 tiles, behind row 0's xbf stream
                    for q in range(2, nq):
                        for cc in range(NCC):
                            nc.sync.dma_start(
                                out=x8sb[cc][q],
                                in_=x8_t[cc][:, :, q * QW:(q + 1) * QW])

                zrow = spool.tile([1, nblk], dt.float32, tag="zrow")
                npc = nj + 1                  # partial columns per c-chunk
                prt = spool.tile([P, NH * npc], dt.float32, tag="prt")

                def pass2_span(lo, width, j, tail=False):
                    # fused multiply+reduce over s-span (needs pbb there):
                    # junk = xbf*pbb (discarded), col = sum(junk)
                    sl = slice(lo, lo + width)
                    for c in range(NH):
                        junk = jpool.tile([P, width], dt.bfloat16, tag="junk",
                                          name="junk")
                        col = prt[:, c * npc + j:c * npc + j + 1]
                        if c in (3, 7):
                            nc.gpsimd.tensor_mul(junk, xbf[c][:, sl],
                                                 pbb[:, sl])
                            if tail:
                                # ScalarE accum (idle in the tail only)
                                sj = jpool.tile([P, width], dt.bfloat16,
                                                tag="sj", name="sj")
                                nc.scalar.activation(
                                    sj, junk,
                                    mybir.ActivationFunctionType.Identity,
                                    accum_out=col)
                            else:
                                nc.vector.tensor_reduce(
                                    col, junk, axis=mybir.AxisListType.X,
                                    op=mybir.AluOpType.add)
                        else:
                            # fused mul+reduce via native TensorScalarPtr
                            # (NOT the custom-ISA tensor_tensor_reduce,
                            # which faults at runtime here)
                            nc.vector.scalar_tensor_tensor(
                                junk, xbf[c][:, sl], 1.0, pbb[:, sl],
                                op0=mybir.AluOpType.mult,
                                op1=mybir.AluOpType.mult,
                                accum_out=col)

                # ---- pass 1 + pipelined softmax-exp + pass-2 chunks ----
                # super-blocks of 2*NB s-columns: one [128, 1024] psum tile
                # per e-chunk (two DoubleRow groups), ONE tanh per e-chunk
                pend = None   # delayed score-matmul args (one e-chunk lag)
                lastrow = (b == b_loc - 1)
                pbq = []      # delayed broadcast: (blk, pe_blk), flushed one
                              # block later so the PE never head-of-line
                              # waits on ScalarE's exp

                def flush_bc():
                    while pbq:
                        pblk, ppe = pbq.pop(0)
                        bc = bcp.tile([P, NB], dt.float32, tag="bc", name="bc")
                        nc.tensor.matmul(bc, lhsT=ones1, rhs=ppe)
                        nc.vector.tensor_copy(
                            pbb[:, pblk * NB:(pblk + 1) * NB], bc)
                        if lastrow:
                            if pblk == 2:
                                pass2_span(0, QP, 0)
                            elif pblk == 3:
                                pass2_span(QP, QP, 1)
                            elif pblk == 5:
                                pass2_span(2 * QP, QP, 2)
                            if pblk >= nblk - 2:
                                pass2_span(pblk * NB, NB,
                                           pblk - (nblk - 2) + nj - 1,
                                           tail=(pblk == nblk - 1))
                        else:
                            if pblk == 3:
                                pass2_span(0, QP, 0)
                            elif pblk == 4:
                                pass2_span(QP, QP, 1)
                            elif pblk == 5:
                                pass2_span(2 * QP, QP, 2)
                            elif pblk == 7:
                                pass2_span(3 * QP, QP, 3)

                for blk in range(nblk):
                    g = b * nblk + blk
                    q = g // (QW // NB)
                    off = (g % (QW // NB)) * NB
                    sc_ps = scp.tile([1, NB], dt.float32, tag="scps", name="scps")
                    for h in range(NH):
                        ps = drp.tile([P, NB], dt.float32, tag="hps", name="hps")
                        for cc in range(NCC):
                            nc.tensor.matmul(
                                ps,
                                lhsT=w8sb[:, h * NCC + cc],
                                rhs=x8sb[cc][q][:, :, off:off + NB],
                                start=(cc == 0), stop=(cc == NCC - 1),
                                perf_mode=DR,
                            )
                        if pend is not None:
                            nc.tensor.matmul(*pend[0], **pend[1])
                        if h == 1:
                            flush_bc()   # prev block's broadcast, mid-stream
                        th = hpool.tile([P, NB], dt.bfloat16, tag="th", name="th")
                        nc.scalar.activation(th, ps,
                                             mybir.ActivationFunctionType.Tanh,
                                             bias=pcb[:, h, b:b + 1],
                                             scale=1.0 / WSCALE)
                        pend = ((sc_ps,), dict(lhsT=wat[:, h:h + 1], rhs=th,
                                               start=(h == 0), stop=(h == NH - 1)))
                    nc.tensor.matmul(*pend[0], **pend[1])
                    pend = None
                    pe_blk = hpool.tile([1, NB], dt.bfloat16, tag="pe", name="pe")
                    nc.scalar.activation(pe_blk, sc_ps,
                                         mybir.ActivationFunctionType.Exp,
                                         accum_out=zrow[:, blk:blk + 1])
                    pbq.append((blk, pe_blk))
                    if blk == nblk - 1:
                        # row's final block: flush now so the row epilogue's
                        # emission order stays a valid dataflow
                        flush_bc()

                # ---- epilogue: combine partials, ship out. Last row on
                # ScalarE (idle in the tail); earlier rows on DVE so their
                # epilogue never queues ahead of the next row's tanh stream.
                ncols = npc if b == b_loc - 1 else nj
                outsb = spool.tile([P, NH], dt.float32, tag=f"outsb{b}",
                                   name=f"outsb{b}")
                if b == b_loc - 1:
                    zsj = spool.tile([1, nblk], dt.float32, tag="zsj")
                    nc.scalar.activation(zsj, zrow,
                                         mybir.ActivationFunctionType.Identity,
                                         accum_out=zout[:, b:b + 1])
                    psj = spool.tile([P, NH, npc], dt.float32, tag="psj")
                    for c in range(NH):
                        nc.scalar.activation(psj[:, c, 0:ncols],
                                             prt[:, c * npc:c * npc + ncols],
                                             mybir.ActivationFunctionType.Identity,
                                             accum_out=outsb[:, c:c + 1])
                else:
                    nc.vector.tensor_reduce(zout[:, b:b + 1], zrow,
                                            axis=mybir.AxisListType.X,
                                            op=mybir.AluOpType.add)
                    for c in range(NH):
                        nc.vector.tensor_reduce(outsb[:, c:c + 1],
                                                prt[:, c * npc:
                                                     c * npc + ncols],
                                                axis=mybir.AxisListType.X,
                                                op=mybir.AluOpType.add)
                nc.sync.dma_start(out=out_r[b], in_=outsb)
            nc.sync.dma_start(out=z_t[:], in_=zout)

    nc.compile()
    return nc


def get_nc(m_loc=M, b_loc=B_LOC, reps=1):
    key = (m_loc, b_loc, reps)
    if key not in _BUILT:
        _BUILT[key] = _build(m_loc, b_loc, reps)
    return _BUILT[key]


def make_in_maps(x, context, W_in, b_in, W_ctx, b_ctx, w_att):
    """Host-side shard + layout prep. All args np full tensors (f32)."""
    x = np.asarray(x, np.float32)
    W_in = np.asarray(W_in, np.float32)
    # w8[k, h*4+cc, i, m] = fp8(16*W_in[h*128+m, cc*256+i*128+k])
    w5 = (W_in * WSCALE).reshape(NH, P, NCC, 2, P)       # [h, m, cc, i, k]
    w8 = np.ascontiguousarray(w5.transpose(4, 0, 2, 3, 1)).astype(FP8)
    w8 = w8.reshape(P, NH * NCC, 2, P)
    # pc[b, e] = context[b] @ W_ctx.T + b_ctx + b_in  (host, f32)
    pc = (np.asarray(context, np.float32) @ np.asarray(W_ctx, np.float32).T
          + np.asarray(b_ctx, np.float32) + np.asarray(b_in, np.float32))
    wat = np.ascontiguousarray(
        np.asarray(w_att, np.float32).reshape(NH, P).T).astype(BF16)
    in_maps = []
    for k in range(N_CORES):
        xs = x[k * B_LOC:(k + 1) * B_LOC].reshape(M, D)
        xT = np.ascontiguousarray(xs.T)                   # [D, M] f32
        # x8[cc, p, i, s] = fp8(xT[cc*256 + i*128 + p, s])
        x8 = np.ascontiguousarray(
            xT.reshape(NCC, 2, P, M).transpose(0, 2, 1, 3)).astype(FP8)
        xbf = xT.astype(BF16)
        pck = pc[k * B_LOC:(k + 1) * B_LOC]               # [b_loc, D]
        pcb = np.ascontiguousarray(
            pck.reshape(B_LOC, NH, P).transpose(2, 1, 0)).astype(np.float32)
        in_maps.append({
            "x8": x8, "xbf": xbf, "w8": w8, "pcb": pcb, "wat": wat,
        })
    return in_maps


def kernel(x, context, W_in, b_in, W_ctx, b_ctx, w_att, b_att):
    # b_att shifts every score equally; softmax is shift-invariant, so it
    # has no effect on the output and is intentionally unused.
    from concourse.bass_utils import run_bass_kernel_spmd

    os.environ.setdefault("BASS_NEVER_TRACE", "1")
    nc = get_nc()
    in_maps = make_in_maps(x, context, W_in, b_in, W_ctx, b_ctx, w_att)
    res = run_bass_kernel_spmd(nc, in_maps, core_ids=list(range(N_CORES)))
    outs = []
    for k in range(N_CORES):
        o = np.asarray(res.results[k]["out"], np.float32)    # [b_loc, D]
        z = np.asarray(res.results[k]["z"], np.float32)      # [1, b_loc]
        outs.append(o / z.reshape(B_LOC, 1))
    return np.concatenate(outs, axis=0)

